# revision 34
# baseline (speedup 1.0000x reference)
"""Distributed Bass kernel for nn_Attention_25744033972479 (Euclidean-bias attention).

Sequence-sharded flash-style attention across 8 TRN2 NeuronCores (v3):
  - core c owns queries [c*nq, (c+1)*nq), nq = n/8
  - K/V projections computed replicated (cross-core collectives measured
    ~110us of dead time on this harness - not worth it)
  - distance bias d2 fused into a 10-row bf16 hi/lo matmul, E = sqrt(d2) once
  - scores in S^T [k, q] layout; per-head-pair row-packed score matmuls
  - E applied per head pair either by one broadcast DVE subtract (fast path,
    exp scale = slope_h; ACT tables are function-keyed so per-head scales are
    free) or, for the last `ident_pairs` pairs, by a -slope_h*I matmul
    accumulated into score PSUM with exp reading PSUM directly (PE/DVE
    balance knob)
  - softmax without max-subtraction (scores bounded); rowsum via ones-col in V
  - normalize via PE broadcast of the rowsum + multi-lane DVE reciprocal
    (no DRAM bounce, no ACT copies -> no ACT table thrash in the main loop)
  - PV emission staggered one chunk behind scores so the in-order PE queue
    never head-of-line blocks on exp
  - PV in out^T [dh, q] layout; output projection fused; host concatenates
"""

import dataclasses
import math
from contextlib import ExitStack

import numpy as np
import ml_dtypes

import concourse.bass as bass
import concourse.bacc as bacc
import concourse.tile as tile
from concourse import mybir
from concourse.bass_utils import run_bass_kernel_spmd

BF = mybir.dt.bfloat16
F32 = mybir.dt.float32
NPBF = ml_dtypes.bfloat16

NCORES = 8
H = 8
DH = 64
D = 512
DC = D // 128  # 4 chunks of the model dim


def get_slopes(n):
    def pow2(n):
        start = 2 ** (-(2 ** (-(math.log2(n) - 3))))
        return [start * start**i for i in range(n)]
    if math.log2(n).is_integer():
        return pow2(n)
    c = 2 ** math.floor(math.log2(n))
    return pow2(c) + get_slopes(2 * c)[0::2][: n - c]


SLOPES = get_slopes(H)  # [0.5, 0.25, ..., 2^-8]


def _bcast2(ap2d):
    """[P, W] -> [P, 2, W] with a stride-0 middle dim."""
    return dataclasses.replace(ap2d, ap=[ap2d.ap[0], [0, 2], ap2d.ap[1]])


def build_kernel(n, nq, ident_pairs=0):
    """Build the per-core SPMD graph. n = total keys, nq = queries per core.

    ident_pairs: number of head pairs (from the end) whose bias-add runs on
    the TensorEngine via -slope*I @ E matmuls (PSUM-accumulated) instead of a
    DVE subtract; their exp reads PSUM directly with scale 1.
    """
    assert n % 128 == 0 and nq % 64 == 0 and n == nq * NCORES
    assert ident_pairs == 0 or nq * 4 >= 2048
    KT = n // 128          # number of 128-key tiles
    KPC = min(KT, 4)       # key tiles per chunk
    NCH = KT // KPC        # chunks per head
    MW = max(nq, D)        # misc psum tile width (one 2KB bank)

    nc = bacc.Bacc("TRN2", target_bir_lowering=False, debug=False, num_devices=NCORES)

    # ---- DRAM parameters (per-core shards prepared on host) ----
    xTf = nc.dram_tensor("xTf", [D, n], BF, kind="ExternalInput").ap()
    xq = nc.dram_tensor("xq", [D, nq], BF, kind="ExternalInput").ap()
    wqT = nc.dram_tensor("wqT", [D, D], BF, kind="ExternalInput").ap()  # head-scaled
    wkT = nc.dram_tensor("wkT", [D, D], BF, kind="ExternalInput").ap()
    wvT = nc.dram_tensor("wvT", [D, D], BF, kind="ExternalInput").ap()
    woT = nc.dram_tensor("woT", [D, D], BF, kind="ExternalInput").ap()
    bqv = nc.dram_tensor("bqv", [128, DC], F32, kind="ExternalInput").ap()
    bkv = nc.dram_tensor("bkv", [128, DC], F32, kind="ExternalInput").ap()
    bov = nc.dram_tensor("bov", [128, DC], F32, kind="ExternalInput").ap()
    # hi/lo bf16 split of the coord augmentation (10 contraction rows)
    augq = nc.dram_tensor("augq", [10, nq], BF, kind="ExternalInput").ap()
    augk = nc.dram_tensor("augk", [10, n], BF, kind="ExternalInput").ap()
    if ident_pairs > 0:
        negi = nc.dram_tensor("negi", [128, 128], BF, kind="ExternalInput").ap()
    outT = nc.dram_tensor("outT", [D, nq], F32, kind="ExternalOutput").ap()

    with tile.TileContext(nc) as tc, ExitStack() as ctx:
        # ---------- persistent pools ----------
        const = ctx.enter_context(tc.tile_pool(name="const", bufs=1))
        big = ctx.enter_context(tc.tile_pool(name="big", bufs=1))
        stage_s = ctx.enter_context(tc.tile_pool(name="stage_s", bufs=2))
        stage_p = ctx.enter_context(tc.tile_pool(name="stage_p", bufs=2))
        small = ctx.enter_context(tc.tile_pool(name="small", bufs=2))
        ps_pair = ctx.enter_context(tc.tile_pool(name="ps_pair", bufs=2, space="PSUM"))
        ps_pv = ctx.enter_context(tc.tile_pool(name="ps_pv", bufs=2, space="PSUM"))
        ps_misc = ctx.enter_context(tc.tile_pool(name="ps_misc", bufs=2, space="PSUM"))

        # resident SBUF tensors (split into per-chunk tiles for fine-grained
        # dependencies: the main loop can start before all readbacks finish)
        NT = n // 512
        ktn = [[big.tile([128, 512], BF, name=f"ktb{cb}_{nt}") for nt in range(NT)]
               for cb in range(DC)]
        VAW = H * 65
        va_tiles = [big.tile([128, KPC * VAW], BF, name=f"vab{c}")
                    for c in range(NCH)]
        et_tiles = [big.tile([128, KPC * nq], BF, name=f"etb{c}")
                    for c in range(NCH)]
        qtb = big.tile([128, DC * nq], BF)       # Q'.T: col = cb*nq + q
        at_tiles = [big.tile([128, nq], BF, name=f"attnT{cb}") for cb in range(DC)]
        va_r = [v.rearrange("p (kt h w) -> p kt h w", h=H, w=65) for v in va_tiles]

        bq_sb = const.tile([128, DC], F32)
        bk_sb = const.tile([128, DC], F32)
        bo_sb = const.tile([128, DC], F32)
        eps_sb = const.tile([128, 1], F32)
        nc.vector.memset(eps_sb[:], 1e-4)
        ones65 = const.tile([65, DH], BF)
        nc.vector.memset(ones65[:], 1.0)
        augq_sb = const.tile([10, nq], BF)
        augk_sb = const.tile([10, n], BF)

        nc.sync.dma_start(out=bq_sb[:], in_=bqv[:, :])
        nc.sync.dma_start(out=bk_sb[:], in_=bkv[:, :])
        nc.sync.dma_start(out=bo_sb[:], in_=bov[:, :])
        nc.sync.dma_start(out=augq_sb[:], in_=augq[:, :])
        nc.sync.dma_start(out=augk_sb[:], in_=augk[:, :])
        if ident_pairs > 0:
            negi_sb = const.tile([128, 128], BF)
            nc.sync.dma_start(out=negi_sb[:], in_=negi[:, :])

        with tc.tile_pool(name="proj", bufs=1) as proj, \
             tc.tile_pool(name="xstream", bufs=3) as xstream:
            xq_sb = proj.tile([128, DC * nq], BF)  # own x.T for Q: col = dc*nq+nl
            wq_sb = proj.tile([128, DC * D], BF)   # col = dc*D + do
            wk_sb = proj.tile([128, DC * D], BF)
            wv_sb = proj.tile([128, DC * D], BF)

            nc.sync.dma_start(out=xq_sb[:],
                              in_=xq.rearrange("(dc p) q -> p dc q", p=128))
            nc.gpsimd.dma_start(out=wk_sb[:],
                                in_=wkT.rearrange("(dc p) d -> p dc d", p=128))
            nc.gpsimd.dma_start(out=wv_sb[:],
                                in_=wvT.rearrange("(dc p) d -> p dc d", p=128))
            nc.gpsimd.dma_start(out=wq_sb[:],
                                in_=wqT.rearrange("(dc p) d -> p dc d", p=128))

            def q_proj(cb):
                ps = ps_misc.tile([128, MW], F32, tag="misc", name=f"qp{cb}")
                for dc in range(DC):
                    nc.tensor.matmul(
                        ps[:, 0:nq],
                        lhsT=wq_sb[:, dc * D + cb * 128: dc * D + (cb + 1) * 128],
                        rhs=xq_sb[:, dc * nq:(dc + 1) * nq],
                        start=(dc == 0), stop=(dc == DC - 1),
                    )
                nc.vector.tensor_scalar_add(qtb[:, cb * nq:(cb + 1) * nq],
                                            ps[:, 0:nq], bq_sb[:, cb:cb + 1])

            def d2_tile(kt):
                # lives in the sc ring: keeps ps_misc free for the K/V chains
                # and keeps ACT on an uninterrupted Sqrt run (no table thrash)
                ps = ps_pair.tile([128, 2 * nq], F32, tag="sc", name=f"d2{kt}")
                nc.tensor.matmul(
                    ps[:, 0:nq],
                    lhsT=augk_sb[:, kt * 128:(kt + 1) * 128],
                    rhs=augq_sb[:, :],
                    start=True, stop=True,
                )
                # +1e-4 guards fp32-cancellation negatives (E err <= 5e-3 at E=0)
                nc.scalar.activation(
                    et_tiles[kt // KPC][:, (kt % KPC) * nq:(kt % KPC + 1) * nq],
                    ps[:, 0:nq],
                    mybir.ActivationFunctionType.Sqrt, bias=eps_sb[:, :])

            # ---- replicated K/V projections over the FULL sequence ----
            # (no collectives; evictions alternate ACT/DVE; accumulation
            #  chains interleaved across two psum slots to hide drains)
            for c in range(NCH):
                nc.vector.memset(va_r[c][:, :, :, 64:65], 1.0)
            # all distance tiles first: one uninterrupted Sqrt run on ACT
            for kt in range(KT):
                d2_tile(kt)
            q_done = {0}
            q_proj(0)
            for nt in range(NT):
                xbt = xstream.tile([128, DC * 512], BF, tag="xbt")
                nc.sync.dma_start(
                    out=xbt[:],
                    in_=bass.AP(tensor=xTf.tensor, offset=xTf.offset + nt * 512,
                                ap=[[n, 128], [128 * n, DC], [1, 512]]))
                # K.T columns for this n-tile: chunk pairs interleaved
                for cb0 in range(0, DC, 2):
                    psA = ps_misc.tile([128, MW], F32, tag="misc", name=f"kpA{nt}_{cb0}")
                    psB = ps_misc.tile([128, MW], F32, tag="misc", name=f"kpB{nt}_{cb0}")
                    for dc in range(DC):
                        for cb, psx in ((cb0, psA), (cb0 + 1, psB)):
                            nc.tensor.matmul(
                                psx[:, 0:512],
                                lhsT=wk_sb[:, dc * D + cb * 128: dc * D + (cb + 1) * 128],
                                rhs=xbt[:, dc * 512:(dc + 1) * 512],
                                start=(dc == 0), stop=(dc == DC - 1),
                            )
                    for cb, psx in ((cb0, psA), (cb0 + 1, psB)):
                        # DVE evictions: ACT stays Sqrt-only in the proj phase
                        # (Identity<->Sqrt alternation thrashes the ACT table
                        # at 1.3us per reload); measured better than ACT here
                        nc.vector.tensor_scalar_add(ktn[cb][nt][:, :],
                                                    psx[:, 0:512],
                                                    bk_sb[:, cb:cb + 1])
                # V rows for this n-tile (4 key tiles of 128), pairs interleaved
                for j0 in range(0, 4, 2):
                    psA = ps_misc.tile([128, MW], F32, tag="misc", name=f"vpA{nt}_{j0}")
                    psB = ps_misc.tile([128, MW], F32, tag="misc", name=f"vpB{nt}_{j0}")
                    for dc in range(DC):
                        for j, psx in ((j0, psA), (j0 + 1, psB)):
                            nc.tensor.matmul(
                                psx[:, 0:D],
                                lhsT=xbt[:, dc * 512 + j * 128: dc * 512 + (j + 1) * 128],
                                rhs=wv_sb[:, dc * D:(dc + 1) * D],
                                start=(dc == 0), stop=(dc == DC - 1),
                            )
                    for j, psx in ((j0, psA), (j0 + 1, psB)):
                        kt = nt * 4 + j
                        dst = va_r[kt // KPC][:, kt % KPC, :, 0:64]
                        srcv = psx[:, 0:D].rearrange("p (h w) -> p h w", w=64)
                        nc.vector.tensor_copy(dst, srcv)
                if nt == 0 and DC > 1:
                    q_proj(1); q_done.add(1)
                elif nt == 1:
                    for cb in range(2, DC):
                        q_proj(cb); q_done.add(cb)
            for cb in range(DC):
                if cb not in q_done:
                    q_proj(cb); q_done.add(cb)

        # ---------- main attention loop over head pairs ----------
        for pr in range(H // 2):
            h1, h2 = 2 * pr, 2 * pr + 1
            cb = pr  # do-chunk holding this head pair
            use_ident = pr >= (H // 2 - ident_pairs)
            pv1 = ps_pv.tile([65, nq], F32, tag="pv")
            pv2 = ps_pv.tile([65, nq], F32, tag="pv")

            def pv_chunk(ch, p_t):
                # PV accumulation (ones column makes row 64 the softmax denom)
                for j in range(KPC):
                    kt = ch * KPC + j
                    for half, h, pv in ((0, h1, pv1), (1, h2, pv2)):
                        nc.tensor.matmul(
                            pv[:],
                            lhsT=va_r[ch][:, j, h, :],
                            rhs=p_t[:, (half * KPC + j) * nq:(half * KPC + j + 1) * nq],
                            start=(kt == 0), stop=(kt == KT - 1),
                        )

            prev_pt = None
            for ch in range(NCH):
                if not use_ident:
                    s_t = stage_s.tile([128, 2 * KPC * nq], BF, tag="sch")
                    s_r = s_t.rearrange("p (s c) -> p s c", s=2)
                p_t = stage_p.tile([128, 2 * KPC * nq], BF, tag="pch")
                p_r = p_t.rearrange("p (s c) -> p s c", s=2)
                for j in range(KPC):
                    kt = ch * KPC + j
                    e_sl = et_tiles[ch][:, j * nq:(j + 1) * nq]
                    sc = ps_pair.tile([128, 2 * nq], F32, tag="sc")
                    # row-packed score matmuls for the head pair
                    klhs = ktn[cb][kt // 4]
                    ko = (kt % 4) * 128
                    nc.tensor.matmul(
                        sc[:, 0:nq],
                        lhsT=klhs[0:64, ko:ko + 128],
                        rhs=qtb[0:64, cb * nq:(cb + 1) * nq],
                        start=True, stop=not use_ident,
                        skip_group_check=use_ident,
                    )
                    nc.tensor.matmul(
                        sc[:, nq:2 * nq],
                        lhsT=klhs[64:128, ko:ko + 128],
                        rhs=qtb[64:128, cb * nq:(cb + 1) * nq],
                        start=True, stop=not use_ident,
                        skip_group_check=use_ident,
                    )
                    if use_ident:
                        # bias via PE: PSUM += -E (Wq pre-scaled by
                        # 1/(8*slope_h)), then exp reads PSUM with scale s_h
                        for half, h in ((0, h1), (1, h2)):
                            nc.tensor.matmul(
                                sc[:, half * nq:(half + 1) * nq],
                                lhsT=negi_sb[:, :],
                                rhs=e_sl,
                                start=False, stop=True,
                                skip_group_check=True,
                            )
                        for half, h in ((0, h1), (1, h2)):
                            nc.scalar.activation(
                                p_r[:, half, j * nq:(j + 1) * nq],
                                sc[:, half * nq:(half + 1) * nq],
                                mybir.ActivationFunctionType.Exp,
                                scale=float(SLOPES[h]),
                            )
                    else:
                        # S' = M' - E for both heads in one DVE op
                        nc.vector.tensor_sub(
                            s_r[:, :, j * nq:(j + 1) * nq],
                            sc.rearrange("p (s c) -> p s c", s=2),
                            _bcast2(e_sl),
                        )
                if not use_ident:
                    # exp (ACT): P = exp(slope_h * S')
                    for half, h in ((0, h1), (1, h2)):
                        nc.scalar.activation(
                            p_t[:, half * KPC * nq:(half + 1) * KPC * nq],
                            s_t[:, half * KPC * nq:(half + 1) * KPC * nq],
                            mybir.ActivationFunctionType.Exp,
                            scale=float(SLOPES[h]),
                        )
                if prev_pt is not None:
                    pv_chunk(ch - 1, prev_pt)
                prev_pt = p_t
            pv_chunk(NCH - 1, prev_pt)

            # ---- normalize: attnT[head rows, q] = pv[0:64] / pv[64] ----
            # rowsum (psum partition 64) -> bf16 sbuf, PE-broadcast to 64
            # partitions, multi-lane reciprocal, then the eviction multiply
            rsb = small.tile([65, 2 * nq], BF, tag="rsb")
            nc.vector.tensor_copy(rsb[64:65, 0:nq], pv1[64:65, :])
            nc.vector.tensor_copy(rsb[64:65, nq:2 * nq], pv2[64:65, :])
            for half, h, pv in ((0, h1, pv1), (1, h2, pv2)):
                psb = ps_misc.tile([128, MW], F32, tag="misc", name=f"nb{pr}_{half}")
                nc.tensor.matmul(
                    psb[0:64, 0:nq],
                    lhsT=ones65[64:65, :],
                    rhs=rsb[64:65, half * nq:(half + 1) * nq],
                    start=True, stop=True,
                )
                rcp = small.tile([64, nq], F32, tag="rcp")
                nc.vector.reciprocal_approx_fast(out=rcp[:], in_=psb[0:64, 0:nq])
                if half == 0:
                    nc.vector.tensor_mul(at_tiles[cb][0:64, :], pv[0:64, :], rcp[:])
                else:
                    odd = small.tile([64, nq], BF, tag="odd")
                    nc.vector.tensor_mul(odd[:], pv[0:64, :], rcp[:])
                    nc.sync.dma_start(out=at_tiles[cb][64:128, :], in_=odd[:])

        # ---------- output projection: out.T = Wo @ attn.T + bo' ----------
        with tc.tile_pool(name="oproj", bufs=1) as oproj, \
             tc.tile_pool(name="ost", bufs=2) as ost:
            wo_sb = oproj.tile([128, DC * D], BF)
            nc.sync.dma_start(out=wo_sb[:],
                              in_=woT.rearrange("(dc p) d -> p dc d", p=128))
            for cb in range(DC):
                ps = ps_misc.tile([128, nq], F32, tag="misc")
                for dc in range(DC):
                    nc.tensor.matmul(
                        ps[:],
                        lhsT=wo_sb[:, dc * D + cb * 128: dc * D + (cb + 1) * 128],
                        rhs=at_tiles[dc][:, :],
                        start=(dc == 0), stop=(dc == DC - 1),
                    )
                fo = ost.tile([128, nq], F32, tag="fo")
                nc.scalar.activation(fo[:], ps[:],
                                     mybir.ActivationFunctionType.Identity,
                                     bias=bo_sb[:, cb:cb + 1])
                nc.sync.dma_start(out=outT[cb * 128:(cb + 1) * 128, :], in_=fo[:])

    nc.compile()
    return nc


def build_kernel2(n, nq):
    """Concurrent-pairs variant: each of two groups runs a DVE-subtract pair
    and a PE-inject (ident) pair at the same time, so the Vector and Tensor
    engines are co-busy instead of alternating bound phases. Group 1 is also
    interleaved chunk-by-chunk with the K/V projection loop (chunk ch only
    needs K/V tiles of n-tile ch). Pairs (0,1) subtract on DVE; pairs (2,3)
    inject -E into score PSUM on the PE, exp reads PSUM.

    PSUM budget (8 banks): scA [128,2nq]f32 x1 buf = 2 banks (sub pair),
    scB [128,nq]f32 x2 bufs = 2 banks (ident pair halves; also shared by the
    d2/Q/KV-proj/normalize/oproj psums), pv [65,nq]f32 x4 bufs = 4 banks.
    """
    assert n % 512 == 0 and nq % 128 == 0 and n == nq * NCORES
    KT = n // 128
    KPC = 4
    NCH = KT // KPC
    nc = bacc.Bacc("TRN2", target_bir_lowering=False, debug=False, num_devices=NCORES)

    xTf = nc.dram_tensor("xTf", [D, n], BF, kind="ExternalInput").ap()
    xq = nc.dram_tensor("xq", [D, nq], BF, kind="ExternalInput").ap()
    wqT = nc.dram_tensor("wqT", [D, D], BF, kind="ExternalInput").ap()
    wkT = nc.dram_tensor("wkT", [D, D], BF, kind="ExternalInput").ap()
    wvT = nc.dram_tensor("wvT", [D, D], BF, kind="ExternalInput").ap()
    woT = nc.dram_tensor("woT", [D, D], BF, kind="ExternalInput").ap()
    bqv = nc.dram_tensor("bqv", [128, DC], F32, kind="ExternalInput").ap()
    bkv = nc.dram_tensor("bkv", [128, DC], F32, kind="ExternalInput").ap()
    bov = nc.dram_tensor("bov", [128, DC], F32, kind="ExternalInput").ap()
    augq = nc.dram_tensor("augq", [10, nq], BF, kind="ExternalInput").ap()
    augk = nc.dram_tensor("augk", [10, n], BF, kind="ExternalInput").ap()
    negi = nc.dram_tensor("negi", [128, 128], BF, kind="ExternalInput").ap()
    outT = nc.dram_tensor("outT", [D, nq], F32, kind="ExternalOutput").ap()

    with tile.TileContext(nc) as tc, ExitStack() as ctx:
        const = ctx.enter_context(tc.tile_pool(name="const", bufs=1))
        big = ctx.enter_context(tc.tile_pool(name="big", bufs=1))
        stage_s = ctx.enter_context(tc.tile_pool(name="stage_s", bufs=2))
        stage_p = ctx.enter_context(tc.tile_pool(name="stage_p", bufs=3))
        small = ctx.enter_context(tc.tile_pool(name="small", bufs=2))
        ps_a = ctx.enter_context(tc.tile_pool(name="ps_a", bufs=1, space="PSUM"))
        ps_b = ctx.enter_context(tc.tile_pool(name="ps_b", bufs=2, space="PSUM"))
        ps_pv = ctx.enter_context(tc.tile_pool(name="ps_pv", bufs=4, space="PSUM"))

        NT = n // 512
        ktn = [[big.tile([128, 512], BF, name=f"ktb{cb}_{nt}") for nt in range(NT)]
               for cb in range(DC)]
        VAW = H * 65
        va_tiles = [big.tile([128, KPC * VAW], BF, name=f"vab{c}")
                    for c in range(NCH)]
        et_tiles = [big.tile([128, KPC * nq], BF, name=f"etb{c}")
                    for c in range(NCH)]
        qtb = big.tile([128, DC * nq], BF)
        at_tiles = [big.tile([128, nq], BF, name=f"attnT{cb}") for cb in range(DC)]
        va_r = [v.rearrange("p (kt h w) -> p kt h w", h=H, w=65) for v in va_tiles]

        bq_sb = const.tile([128, DC], F32)
        bk_sb = const.tile([128, DC], F32)
        bo_sb = const.tile([128, DC], F32)
        eps_sb = const.tile([128, 1], F32)
        nc.vector.memset(eps_sb[:], 1e-4)
        ones65 = const.tile([65, DH], BF)
        nc.vector.memset(ones65[:], 1.0)
        augq_sb = const.tile([10, nq], BF)
        augk_sb = const.tile([10, n], BF)
        negi_sb = const.tile([128, 128], BF)

        nc.sync.dma_start(out=augq_sb[:], in_=augq[:, :])
        nc.sync.dma_start(out=augk_sb[:], in_=augk[:, :])
        nc.sync.dma_start(out=bq_sb[:], in_=bqv[:, :])
        nc.sync.dma_start(out=bk_sb[:], in_=bkv[:, :])
        nc.sync.dma_start(out=bo_sb[:], in_=bov[:, :])
        nc.sync.dma_start(out=negi_sb[:], in_=negi[:, :])

        # ---------- shared helpers ----------
        def pv_chunk(pr, pvp, ch, p_t):
            h1, h2 = 2 * pr, 2 * pr + 1
            for j in range(KPC):
                kt = ch * KPC + j
                for half, h, pv in ((0, h1, pvp[0]), (1, h2, pvp[1])):
                    nc.tensor.matmul(
                        pv[:],
                        lhsT=va_r[ch][:, j, h, :],
                        rhs=p_t[:, (half * KPC + j) * nq:(half * KPC + j + 1) * nq],
                        start=(kt == 0), stop=(kt == KT - 1),
                    )

        def normalize(pr, pvp):
            pv1, pv2 = pvp
            rsb = small.tile([65, 2 * nq], BF, tag="rsb", name=f"rsb{pr}")
            nc.vector.tensor_copy(rsb[64:65, 0:nq], pv1[64:65, :])
            nc.vector.tensor_copy(rsb[64:65, nq:2 * nq], pv2[64:65, :])
            for half, pv in ((0, pv1), (1, pv2)):
                psb = ps_b.tile([128, nq], F32, tag="b", name=f"nb{pr}_{half}")
                nc.tensor.matmul(
                    psb[0:64, 0:nq],
                    lhsT=ones65[64:65, :],
                    rhs=rsb[64:65, half * nq:(half + 1) * nq],
                    start=True, stop=True,
                )
                rcp = small.tile([64, nq], F32, tag="rcp", name=f"rcp{pr}_{half}")
                nc.vector.reciprocal_approx_fast(out=rcp[:], in_=psb[0:64, 0:nq])
                if half == 0:
                    nc.vector.tensor_mul(at_tiles[pr][0:64, :], pv[0:64, :], rcp[:])
                else:
                    odd = small.tile([64, nq], BF, tag="odd", name=f"odd{pr}")
                    nc.vector.tensor_mul(odd[:], pv[0:64, :], rcp[:])
                    nc.sync.dma_start(out=at_tiles[pr][64:128, :], in_=odd[:])

        def new_group(prA, prB):
            return {
                "prA": prA, "prB": prB,
                "pvA": (ps_pv.tile([65, nq], F32, tag="pv", name=f"pva{prA}"),
                        ps_pv.tile([65, nq], F32, tag="pv", name=f"pvb{prA}")),
                "pvB": (ps_pv.tile([65, nq], F32, tag="pv", name=f"pva{prB}"),
                        ps_pv.tile([65, nq], F32, tag="pv", name=f"pvb{prB}")),
                "prev": None,
            }

        def group_chunk(g, ch, identB=True):
            prA, prB = g["prA"], g["prB"]
            hA1, hA2 = 2 * prA, 2 * prA + 1
            hB1, hB2 = 2 * prB, 2 * prB + 1
            s_t = stage_s.tile([128, 2 * KPC * nq], BF, tag="sch",
                               name=f"st{prA}_{ch}")
            s_r = s_t.rearrange("p (s c) -> p s c", s=2)
            p_tA = stage_p.tile([128, 2 * KPC * nq], BF, tag="pch",
                                name=f"ptA{prA}_{ch}")
            p_tB = stage_p.tile([128, 2 * KPC * nq], BF, tag="pch",
                                name=f"ptB{prB}_{ch}")
            p_rB = p_tB.rearrange("p (s c) -> p s c", s=2)
            if not identB:
                s_tB = stage_s.tile([128, 2 * KPC * nq], BF, tag="schB",
                                    name=f"stB{prB}_{ch}")
                s_rB = s_tB.rearrange("p (s c) -> p s c", s=2)
            for j in range(KPC):
                kt = ch * KPC + j
                ko = j * 128
                e_sl = et_tiles[ch][:, j * nq:(j + 1) * nq]
                # --- sub pair A: one 4KB psum tile, one DVE subtract ---
                scA = ps_a.tile([128, 2 * nq], F32, tag="a", name=f"scA{prA}_{kt}")
                klhsA = ktn[prA][ch]
                nc.tensor.matmul(scA[:, 0:nq],
                                 lhsT=klhsA[0:64, ko:ko + 128],
                                 rhs=qtb[0:64, prA * nq:(prA + 1) * nq],
                                 start=True, stop=True)
                nc.tensor.matmul(scA[:, nq:2 * nq],
                                 lhsT=klhsA[64:128, ko:ko + 128],
                                 rhs=qtb[64:128, prA * nq:(prA + 1) * nq],
                                 start=True, stop=True)
                nc.vector.tensor_sub(
                    s_r[:, :, j * nq:(j + 1) * nq],
                    scA.rearrange("p (s c) -> p s c", s=2),
                    _bcast2(e_sl),
                )
                # --- pair B: per-half 2KB psum tiles ---
                klhsB = ktn[prB][ch]
                for half, h, qlo in ((0, hB1, 0), (1, hB2, 64)):
                    scB = ps_b.tile([128, nq], F32, tag="b",
                                    name=f"scB{prB}_{kt}_{half}")
                    nc.tensor.matmul(scB[:, :],
                                     lhsT=klhsB[qlo:qlo + 64, ko:ko + 128],
                                     rhs=qtb[qlo:qlo + 64, prB * nq:(prB + 1) * nq],
                                     start=True, stop=not identB,
                                     skip_group_check=identB)
                    if identB:
                        # PE E-inject, exp reads PSUM
                        nc.tensor.matmul(scB[:, :],
                                         lhsT=negi_sb[:, :],
                                         rhs=e_sl,
                                         start=False, stop=True,
                                         skip_group_check=True)
                        nc.scalar.activation(
                            p_rB[:, half, j * nq:(j + 1) * nq], scB[:, :],
                            mybir.ActivationFunctionType.Exp,
                            scale=float(SLOPES[h]),
                        )
                    else:
                        nc.vector.tensor_sub(
                            s_rB[:, half, j * nq:(j + 1) * nq],
                            scB[:, :], e_sl)
            for half, h in ((0, hA1), (1, hA2)):
                nc.scalar.activation(
                    p_tA[:, half * KPC * nq:(half + 1) * KPC * nq],
                    s_t[:, half * KPC * nq:(half + 1) * KPC * nq],
                    mybir.ActivationFunctionType.Exp,
                    scale=float(SLOPES[h]),
                )
            if not identB:
                for half, h in ((0, hB1), (1, hB2)):
                    nc.scalar.activation(
                        p_tB[:, half * KPC * nq:(half + 1) * KPC * nq],
                        s_tB[:, half * KPC * nq:(half + 1) * KPC * nq],
                        mybir.ActivationFunctionType.Exp,
                        scale=float(SLOPES[h]),
                    )
            if g["prev"] is not None:
                pA, pB = g["prev"]
                pv_chunk(prA, g["pvA"], ch - 1, pA)
                pv_chunk(prB, g["pvB"], ch - 1, pB)
            g["prev"] = (p_tA, p_tB)

        def group_tail(g):
            pA, pB = g["prev"]
            pv_chunk(g["prA"], g["pvA"], NCH - 1, pA)
            pv_chunk(g["prB"], g["pvB"], NCH - 1, pB)
            normalize(g["prA"], g["pvA"])
            normalize(g["prB"], g["pvB"])

        # ---------- projections interleaved with group 1 ----------
        with tc.tile_pool(name="proj", bufs=1) as proj, \
             tc.tile_pool(name="xstream", bufs=2) as xstream:
            xq_sb = proj.tile([128, DC * nq], BF)
            wq_sb = proj.tile([128, DC * D], BF)
            wk_sb = proj.tile([128, DC * D], BF)
            wv_sb = proj.tile([128, DC * D], BF)

            nc.sync.dma_start(out=xq_sb[:],
                              in_=xq.rearrange("(dc p) q -> p dc q", p=128))
            nc.gpsimd.dma_start(out=wk_sb[:],
                                in_=wkT.rearrange("(dc p) d -> p dc d", p=128))
            nc.gpsimd.dma_start(out=wq_sb[:],
                                in_=wqT.rearrange("(dc p) d -> p dc d", p=128))
            nc.gpsimd.dma_start(out=wv_sb[:],
                                in_=wvT.rearrange("(dc p) d -> p dc d", p=128))

            for c in range(NCH):
                nc.vector.memset(va_r[c][:, :, :, 64:65], 1.0)

            def d2_tile(kt):
                ps = ps_b.tile([128, nq], F32, tag="b", name=f"d2{kt}")
                nc.tensor.matmul(
                    ps[:, :],
                    lhsT=augk_sb[:, kt * 128:(kt + 1) * 128],
                    rhs=augq_sb[:, :],
                    start=True, stop=True,
                )
                nc.scalar.activation(
                    et_tiles[kt // KPC][:, (kt % KPC) * nq:(kt % KPC + 1) * nq],
                    ps[:, :],
                    mybir.ActivationFunctionType.Sqrt, bias=eps_sb[:, :])

            def q_proj(cb):
                ps = ps_b.tile([128, nq], F32, tag="b", name=f"qp{cb}")
                for dc in range(DC):
                    nc.tensor.matmul(
                        ps[:, :],
                        lhsT=wq_sb[:, dc * D + cb * 128: dc * D + (cb + 1) * 128],
                        rhs=xq_sb[:, dc * nq:(dc + 1) * nq],
                        start=(dc == 0), stop=(dc == DC - 1),
                    )
                nc.vector.tensor_scalar_add(qtb[:, cb * nq:(cb + 1) * nq],
                                            ps[:, :], bq_sb[:, cb:cb + 1])

            # distance tiles + Q first (uninterrupted Sqrt run on ACT)
            for kt in range(KT):
                d2_tile(kt)
                if kt % 8 == 7:
                    q_proj(kt // 8)

            def proj_nt(nt):
                xbt = xstream.tile([128, DC * 512], BF, tag="xbt")
                nc.sync.dma_start(
                    out=xbt[:],
                    in_=bass.AP(tensor=xTf.tensor, offset=xTf.offset + nt * 512,
                                ap=[[n, 128], [128 * n, DC], [1, 512]]))
                for cb0 in range(0, DC, 2):
                    psA = ps_b.tile([128, nq], F32, tag="b", name=f"kpA{nt}_{cb0}")
                    psB = ps_b.tile([128, nq], F32, tag="b", name=f"kpB{nt}_{cb0}")
                    for dc in range(DC):
                        for cb, psx in ((cb0, psA), (cb0 + 1, psB)):
                            nc.tensor.matmul(
                                psx[:, 0:512],
                                lhsT=wk_sb[:, dc * D + cb * 128: dc * D + (cb + 1) * 128],
                                rhs=xbt[:, dc * 512:(dc + 1) * 512],
                                start=(dc == 0), stop=(dc == DC - 1),
                            )
                    for cb, psx in ((cb0, psA), (cb0 + 1, psB)):
                        nc.vector.tensor_scalar_add(ktn[cb][nt][:, :],
                                                    psx[:, 0:512],
                                                    bk_sb[:, cb:cb + 1])
                for j0 in range(0, 4, 2):
                    psA = ps_b.tile([128, nq], F32, tag="b", name=f"vpA{nt}_{j0}")
                    psB = ps_b.tile([128, nq], F32, tag="b", name=f"vpB{nt}_{j0}")
                    for dc in range(DC):
                        for j, psx in ((j0, psA), (j0 + 1, psB)):
                            nc.tensor.matmul(
                                psx[:, 0:D],
                                lhsT=xbt[:, dc * 512 + j * 128: dc * 512 + (j + 1) * 128],
                                rhs=wv_sb[:, dc * D:(dc + 1) * D],
                                start=(dc == 0), stop=(dc == DC - 1),
                            )
                    for j, psx in ((j0, psA), (j0 + 1, psB)):
                        kt = nt * 4 + j
                        dst = va_r[kt // KPC][:, kt % KPC, :, 0:64]
                        srcv = psx[:, 0:D].rearrange("p (h w) -> p h w", w=64)
                        nc.vector.tensor_copy(dst, srcv)

            g1 = new_group(0, 2)
            for nt in range(NT):
                proj_nt(nt)
                group_chunk(g1, nt)
            group_tail(g1)

        # group 2 runs sub+sub: it is DVE-paced and keeps the inject matmuls
        # (PE is the saturated engine) down to one pair total
        g2 = new_group(1, 3)
        for ch in range(NCH):
            group_chunk(g2, ch, identB=False)
        group_tail(g2)

        # ---------- output projection ----------
        with tc.tile_pool(name="oproj", bufs=1) as oproj, \
             tc.tile_pool(name="ost", bufs=2) as ost:
            wo_sb = oproj.tile([128, DC * D], BF)
            nc.sync.dma_start(out=wo_sb[:],
                              in_=woT.rearrange("(dc p) d -> p dc d", p=128))
            for cb in range(DC):
                ps = ps_b.tile([128, nq], F32, tag="b", name=f"op{cb}")
                for dc in range(DC):
                    nc.tensor.matmul(
                        ps[:],
                        lhsT=wo_sb[:, dc * D + cb * 128: dc * D + (cb + 1) * 128],
                        rhs=at_tiles[dc][:, :],
                        start=(dc == 0), stop=(dc == DC - 1),
                    )
                fo = ost.tile([128, nq], F32, tag="fo")
                nc.scalar.activation(fo[:], ps[:],
                                     mybir.ActivationFunctionType.Identity,
                                     bias=bo_sb[:, cb:cb + 1])
                nc.sync.dma_start(out=outT[cb * 128:(cb + 1) * 128, :], in_=fo[:])

    nc.compile()
    return nc


def prep_inputs(x, coords, Wq, bq, Wk, bk, Wv, bv, Wo, bo, n, nq, ident_pairs=0):
    """Host-side shard/layout prep. Returns per-core input maps."""
    f32 = np.float32
    x2 = np.asarray(x, f32).reshape(n, D)
    c2 = np.asarray(coords, f32).reshape(n, 2)
    xT = np.ascontiguousarray(x2.T)  # [D, n]

    # per-head scaling of Wq: q'_h = q_h / (8 * slope_h); exp scale = slope_h
    # restores scores = qk/8 - slope_h*E for both the subtract and the
    # identity-inject path (the slope multiplies ONLY the bias term)
    qscale = np.repeat(np.array([1.0 / (8.0 * s) for s in SLOPES], f32), DH)  # [D]
    wqT = np.ascontiguousarray((np.asarray(Wq, f32) * qscale[:, None]).T)  # [di, do]
    wkT = np.ascontiguousarray(np.asarray(Wk, f32).T)
    wvT = np.ascontiguousarray(np.asarray(Wv, f32).T)
    woT = np.ascontiguousarray(np.asarray(Wo, f32).T)
    bqs = np.asarray(bq, f32) * qscale
    bos = np.asarray(bo, f32) + np.asarray(Wo, f32) @ np.asarray(bv, f32)

    def cvec(v):  # [512] -> [128, 4]: col cb = chunk, row p = within-chunk index
        return np.ascontiguousarray(np.asarray(v, f32).reshape(DC, 128).T)

    def hilo(v):
        hi = v.astype(NPBF)
        lo = (v - hi.astype(f32)).astype(NPBF)
        return hi, lo

    # d2 = |a|^2 + |b|^2 - 2(ax*bx + ay*by), bf16 hi/lo split (10 rows):
    # products of bf16 pairs are exact in the fp32 PSUM accumulate.
    one_n, one_q = np.ones(n, f32).astype(NPBF), np.ones(nq, f32).astype(NPBF)
    b2h, b2l = hilo((c2 * c2).sum(1))
    bxh, bxl = hilo(c2[:, 0])
    byh, byl = hilo(c2[:, 1])
    augk = np.stack([one_n, one_n, b2h, b2l, bxh, bxl, bxh, byh, byl, byh])
    negi = (-np.eye(128)).astype(NPBF)

    common = {
        "wqT": wqT.astype(NPBF), "wkT": wkT.astype(NPBF),
        "wvT": wvT.astype(NPBF), "woT": woT.astype(NPBF),
        "bqv": cvec(bqs), "bkv": cvec(np.asarray(bk, f32)), "bov": cvec(bos),
        "augk": np.ascontiguousarray(augk),
    }
    if ident_pairs > 0:
        common["negi"] = negi
    common["xTf"] = np.ascontiguousarray(xT).astype(NPBF)
    in_maps = []
    for c in range(NCORES):
        sl = slice(c * nq, (c + 1) * nq)
        a = c2[sl]
        a2h, a2l = hilo((a * a).sum(1))
        mxh, mxl = hilo(-2 * a[:, 0])
        myh, myl = hilo(-2 * a[:, 1])
        augq = np.stack([a2h, a2l, one_q, one_q, mxh, mxh, mxl, myh, myh, myl])
        m = dict(common)
        m["xq"] = np.ascontiguousarray(xT[:, sl]).astype(NPBF)
        m["augq"] = np.ascontiguousarray(augq)
        in_maps.append(m)
    return in_maps


_CACHE = {}


def _get_kernel(n, nq, ident_pairs, concurrent):
    key = (n, nq, ident_pairs, concurrent)
    if key not in _CACHE:
        if concurrent:
            _CACHE[key] = build_kernel2(n, nq)
        else:
            _CACHE[key] = build_kernel(n, nq, ident_pairs)
    return _CACHE[key]


def kernel(x, coords, Wq, bq, Wk, bk, Wv, bv, Wo, bo, _trace=False, _ident_pairs=1,
           _concurrent=False):
    b, n, d = x.shape
    assert b == 1 and d == D
    nq = n // NCORES
    nc = _get_kernel(n, nq, _ident_pairs, _concurrent)
    in_maps = prep_inputs(x, coords, Wq, bq, Wk, bk, Wv, bv, Wo, bo, n, nq,
                          ident_pairs=(2 if _concurrent else _ident_pairs))
    res = None
    for attempt in range(3):
        try:
            res = run_bass_kernel_spmd(nc, in_maps, core_ids=list(range(NCORES)),
                                       trace=_trace)
            break
        except Exception:
            # transient NRT_EXEC_UNIT_UNRECOVERABLE faults have been observed
            # on this tunnel; back off and retry on a clean execution
            if attempt == 2:
                raise
            import time
            time.sleep(5)
    out = np.empty((n, D), np.float32)
    for c in range(NCORES):
        out[c * nq:(c + 1) * nq, :] = res.results[c]["outT"].T
    if _trace:
        kernel._last = res
    return out[None]
